# revision 2
# baseline (speedup 1.0000x reference)
"""Trainium2 Bass kernel for nn_BERT_tensor — instruction-minimized redesign.

Strategy vs baseline:
  - Data-parallel over batch: 4 seqs (800 tokens) per core x 8 cores.
  - TN (MPO) contraction folded into dense QKV weights on host.
  - Everything dim-major [256, 800]; NO PE transposes anywhere.
  - Attention computed transposed: scoresT[k,q] = K^T Q per (seq, head) with
    pad-mask as per-partition activation bias; exp in bf16 (fp32-range
    exponent, no max-subtraction needed: |scores| <= ~35); softmax
    denominator via ones-vector matmul (partition reduction on PE);
    normalization folded into the PSUM->SBUF move of ctx.
  - LayerNorm dim-major: mean/var via ones-matmul partition reductions,
    per-token stats broadcast back via rank-1 matmul; gamma/beta are
    per-partition scalars (free with ACT bias / DVE scalar ops).
  - Single fp16 h state (no separate f32 residual copy).
  - All weights host-prepacked so every DMA is contiguous; 8 DMAs/layer.
"""
import numpy as np
from contextlib import ExitStack

import concourse.bass as bass
import concourse.bacc as bacc
import concourse.tile as tile
import concourse.mybir as mybir
import concourse.bass_isa as bass_isa
from concourse.bass_utils import run_bass_kernel_spmd

dt = mybir.dt
AF = mybir.ActivationFunctionType
ALU = mybir.AluOpType

B, S, D = 32, 200, 256
H, DFF, VOCAB, L, TD = 6, 1024, 3500, 8, 2
N_CORES = 8
BS = B // N_CORES            # 4 seqs per core
T = BS * S                   # 800 tokens per core
KT = D // 128                # 2 tiles over emb dim
NQK = 24                     # m-tiles over Q|K outdim (3072)
NMID = DFF // 128            # 8 tiles over ffn hidden
CH = [(0, 512), (512, 288)]  # bank-aligned free chunks of 800
CH3 = [(0, 512), (512, 512), (1024, 176)]  # chunks of 1200
SEQT = [(0, 128), (128, 72)]
EPS = 1e-6

import os
L_RUN = int(os.environ.get("BERT_L_RUN", str(L)))
REP = int(os.environ.get("BERT_REP", "1"))

f16 = dt.float16
bf16 = dt.bfloat16
f32 = dt.float32

_CACHE = {}


def _build_program():
    nc = bacc.Bacc("TRN2", target_bir_lowering=False, debug=False,
                   num_devices=N_CORES)
    inp = {}

    def din(name, shape, dty):
        inp[name] = nc.dram_tensor(name, list(shape), dty, kind="ExternalInput").ap()
        return inp[name]

    h0_d = din("h0", [KT, 128, T], f16)
    mask_d = din("maskT", [128, 2 * BS], f32)
    wqk_d = din("wqk", [L, KT, 128, 2 * H * D], f16)
    wv_d = din("wv", [L, KT, 128, H * D], f16)
    ow_d = din("ow", [L, 128, 12, D], f16)
    ff1_d = din("ff1", [L, 128, KT, DFF], f16)
    ff2_d = din("ff2", [L, 128, NMID, D], f16)
    sm_d = din("smalls", [L, 128, 44], f32)
    out_d = nc.dram_tensor("out", [D, T], f16, kind="ExternalOutput").ap()

    # smalls column map
    C_BQK, C_OBE, C_F1B, C_F2B = 0, 24, 26, 34
    C_G1, C_B1, C_G2, C_B2 = 36, 38, 40, 42

    with tile.TileContext(nc) as tc:
        with ExitStack() as ctx:
            cpool = ctx.enter_context(tc.tile_pool(name="const", bufs=1))
            wpool = ctx.enter_context(tc.tile_pool(name="weights", bufs=1))
            apool = ctx.enter_context(tc.tile_pool(name="acts", bufs=1))
            psmm = ctx.enter_context(tc.tile_pool(name="psmm", bufs=2, space="PSUM"))

            mask_t = cpool.tile([128, 2 * BS], f32, tag="mask", name="mask_t")
            nc.sync.dma_start(mask_t[:], mask_d[:])
            eps_t = cpool.tile([128, 1], f32, tag="eps", name="eps_t")
            nc.vector.memset(eps_t[:], EPS)

            def mmslot(name, shape=(128, 1536), dty=f32):
                return psmm.tile(list(shape), dty, tag="mm", name=name)

            def arslot(name):
                return apool.tile([128, H * S], f32, tag="ar", bufs=2, name=name)

            for rep in range(REP):
              h16 = []
              for kt in range(KT):
                  t = apool.tile([128, T], f16, tag="h", bufs=4,
                                 name=f"h_init{rep}_{kt}")
                  nc.sync.dma_start(t[:], h0_d[kt])
                  h16.append(t)

              for l in range(L_RUN):
                # ---- layer weights (contiguous DMAs) ----
                wqk_t = []
                for kt in range(KT):
                    t = wpool.tile([128, 2 * H * D], f16, tag=f"wqk{kt}", bufs=1,
                                   name=f"wqk{rep}_{l}_{kt}")
                    nc.sync.dma_start(t[:], wqk_d[l, kt])
                    wqk_t.append(t)
                wv_t = []
                for kt in range(KT):
                    t = wpool.tile([128, H * D], f16, tag=f"wv{kt}", bufs=2,
                                   name=f"wv{rep}_{l}_{kt}")
                    nc.sync.dma_start(t[:], wv_d[l, kt])
                    wv_t.append(t)
                ow_t = wpool.tile([128, 12, D], f16, tag="ow", bufs=2,
                                  name=f"ow{rep}_{l}")
                nc.sync.dma_start(ow_t[:], ow_d[l])
                ff1_t = wpool.tile([128, KT, DFF], f16, tag="ff1", bufs=2,
                                   name=f"ff1{rep}_{l}")
                nc.sync.dma_start(ff1_t[:], ff1_d[l])
                ff2_t = wpool.tile([128, NMID, D], f16, tag="ff2", bufs=2,
                                   name=f"ff2{rep}_{l}")
                nc.sync.dma_start(ff2_t[:], ff2_d[l])
                sm = wpool.tile([128, 44], f32, tag="sm", bufs=2,
                                name=f"sm{rep}_{l}")
                nc.sync.dma_start(sm[:], sm_d[l])

                # ---- QKV (Q|K dim-major fp16 [3072, 800]) ----
                qk = []
                for m in range(NQK):
                    ps = mmslot(f"psqk{rep}_{l}_{m}")
                    for kt in range(KT):
                        for (o, w) in CH:
                            nc.tensor.matmul(
                                ps[:, o:o + w],
                                wqk_t[kt][:, m * 128:(m + 1) * 128],
                                h16[kt][:, o:o + w],
                                start=(kt == 0), stop=(kt == KT - 1))
                    qt = apool.tile([128, T], f16, tag="qk", bufs=NQK,
                                    name=f"qk{rep}_{l}_{m}")
                    nc.scalar.activation(qt[:], ps[:, 0:T], AF.Identity,
                                         bias=sm[:, C_BQK + m:C_BQK + m + 1])
                    qk.append(qt)

                # ---- attention per sequence ----
                ctxA = apool.tile([128, 12, T], f16, tag="ctx", bufs=1,
                                  name=f"ctx{rep}_{l}")
                for b in range(BS):
                    # V token-major bf16 [ts, 1536] per seq-tile
                    vt = []
                    for ti, (to, ts) in enumerate(SEQT):
                        ps = mmslot(f"psv{rep}_{l}_{b}_{ti}")
                        for kt in range(KT):
                            for c in range(3):
                                nc.tensor.matmul(
                                    ps[0:ts, c * 512:(c + 1) * 512],
                                    h16[kt][:, b * S + to:b * S + to + ts],
                                    wv_t[kt][:, c * 512:(c + 1) * 512],
                                    start=(kt == 0), stop=(kt == KT - 1))
                        v = apool.tile([128, H * D], bf16, tag="v", bufs=4,
                                       name=f"v{rep}_{l}_{b}_{ti}")
                        nc.scalar.activation(v[0:ts, :], ps[0:ts, 0:H * D], AF.Copy)
                        vt.append(v)

                    # scoresT + exp (bf16), head-packed PSUM [128, 6, 256]
                    expT = []
                    for ti, (ko, ks) in enumerate(SEQT):
                        ps = psmm.tile([128, H, 256], f32, tag="mm",
                                       name=f"pssc{rep}_{l}_{b}_{ti}")
                        for h in range(H):
                            for kt in range(KT):
                                nc.tensor.matmul(
                                    ps[0:ks, h, 0:S],
                                    qk[12 + h * KT + kt][:, b * S + ko:b * S + ko + ks],
                                    qk[h * KT + kt][:, b * S:(b + 1) * S],
                                    start=(kt == 0), stop=(kt == KT - 1))
                        e = apool.tile([128, H, S], bf16, tag="expT", bufs=4,
                                       name=f"expT{rep}_{l}_{b}_{ti}")
                        nc.scalar.activation(
                            e[0:ks, :, :], ps[0:ks, :, 0:S], AF.Exp,
                            bias=mask_t[0:ks, b * 2 + ti:b * 2 + ti + 1])
                        expT.append(e)

                    # softmax denominators via gpsimd partition reductions
                    ar0 = arslot(f"ar0{rep}_{l}_{b}")
                    ar1 = arslot(f"ar1{rep}_{l}_{b}")
                    nc.gpsimd.partition_all_reduce(
                        ar0[:, :], expT[0][:, :, :], channels=128,
                        reduce_op=bass_isa.ReduceOp.add)
                    nc.gpsimd.partition_all_reduce(
                        ar1[0:72, :], expT[1][0:72, :, :], channels=72,
                        reduce_op=bass_isa.ReduceOp.add)
                    rs = apool.tile([1, H * S], f32, tag="rs", bufs=1,
                                    name=f"rs{rep}_{l}_{b}")
                    nc.vector.tensor_tensor(rs[0:1, :], ar0[0:1, :], ar1[0:1, :],
                                            op=ALU.add)
                    nc.vector.reciprocal(rs[0:1, :], rs[0:1, :])
                    rsb = apool.tile([128, H * S], f32, tag="rsb", bufs=1,
                                     name=f"rsb{rep}_{l}_{b}")
                    nc.gpsimd.partition_broadcast(rsb[:, :], rs[0:1, :])

                    # ctx dim-major, packed [128, 6, 256] per 3-head half;
                    # softmax normalization fused into the PSUM->SBUF move
                    for half in range(2):
                        pc = psmm.tile([128, 6, 256], f32, tag="mm",
                                       name=f"psctx{rep}_{l}_{b}_{half}")
                        for j in range(6):
                            h, dtile = half * 3 + j // 2, j % 2
                            for ti, (ko, ks) in enumerate(SEQT):
                                nc.tensor.matmul(
                                    pc[:, j, 0:S],
                                    vt[ti][0:ks, h * D + dtile * 128:
                                           h * D + (dtile + 1) * 128],
                                    expT[ti][0:ks, h, :],
                                    start=(ti == 0), stop=(ti == 1))
                        for dtile in range(2):
                            nc.vector.tensor_tensor(
                                ctxA[:, half * 6 + dtile:half * 6 + 6:2,
                                     b * S:(b + 1) * S],
                                pc[:, dtile:6:2, 0:S],
                                rsb[:, half * 3 * S:(half + 1) * 3 * S],
                                op=ALU.mult)

                # ---- out projection + residual -> x16 ----
                x16 = []
                for d2 in range(KT):
                    psA = psmm.tile([128, 512], f32, tag="mm",
                                    name=f"psoA{rep}_{l}_{d2}")
                    psB = psmm.tile([128, 512], f32, tag="mm",
                                    name=f"psoB{rep}_{l}_{d2}")
                    pso = [psA, psB]
                    for ct in range(12):
                        for ci, (o, w) in enumerate(CH):
                            nc.tensor.matmul(
                                pso[ci][:, 0:w],
                                ow_t[:, ct, d2 * 128:(d2 + 1) * 128],
                                ctxA[:, ct, o:o + w],
                                start=(ct == 0), stop=(ct == 11))
                    xt = apool.tile([128, T], f16, tag="x", bufs=4,
                                    name=f"x{rep}_{l}_{d2}")
                    for ci, (o, w) in enumerate(CH):
                        nc.vector.scalar_tensor_tensor(
                            xt[:, o:o + w], pso[ci][:, 0:w],
                            sm[:, C_OBE + d2:C_OBE + d2 + 1], h16[d2][:, o:o + w],
                            op0=ALU.add, op1=ALU.add)
                    x16.append(xt)

                def layer_norm(xt, gc, bc, tag, otag, obufs):
                    """dim-major LN: xt 2 fp16 [128,800] tiles -> 2 fp16 tiles."""
                    # -mean
                    # channel sums of x via gpsimd all-reduce (broadcast out)
                    arm = []
                    for kt in range(KT):
                        a = arslot(f"{tag}arm{rep}_{l}_{kt}")
                        nc.gpsimd.partition_all_reduce(
                            a[:, 0:T], xt[kt][:, :], channels=128,
                            reduce_op=bass_isa.ReduceOp.add)
                        arm.append(a)
                    xc, sq = [], []
                    for kt in range(KT):
                        t1 = apool.tile([128, T], f16, tag="lnxc", bufs=4,
                                        name=f"{tag}t1{rep}_{l}_{kt}")
                        nc.vector.scalar_tensor_tensor(
                            t1[:], arm[0][:, 0:T], -1.0 / D, xt[kt][:],
                            op0=ALU.mult, op1=ALU.add)
                        c = apool.tile([128, T], f16, tag="lnxc", bufs=4,
                                       name=f"{tag}xc{rep}_{l}_{kt}")
                        nc.vector.scalar_tensor_tensor(
                            c[:], arm[1][:, 0:T], -1.0 / D, t1[:],
                            op0=ALU.mult, op1=ALU.add)
                        xc.append(c)
                        s = apool.tile([128, T], f16, tag="lnsq", bufs=2,
                                       name=f"{tag}sq{rep}_{l}_{kt}")
                        nc.scalar.activation(s[:], c[:], AF.Square)
                        sq.append(s)
                    # rstd = 1/sqrt(var+eps), broadcast across partitions
                    arv = []
                    for kt in range(KT):
                        a = arslot(f"{tag}arv{rep}_{l}_{kt}")
                        nc.gpsimd.partition_all_reduce(
                            a[:, 0:T], sq[kt][:, :], channels=128,
                            reduce_op=bass_isa.ReduceOp.add)
                        arv.append(a)
                    var = apool.tile([128, T], f32, tag="lnvar", bufs=1,
                                     name=f"{tag}var{rep}_{l}")
                    nc.vector.tensor_tensor(var[:], arv[0][:, 0:T], arv[1][:, 0:T],
                                            op=ALU.add)
                    nc.scalar.activation(var[:], var[:], AF.Sqrt, scale=1.0 / D,
                                         bias=eps_t[:, :])
                    nc.vector.reciprocal(var[:], var[:])
                    outs = []
                    for kt in range(KT):
                        tmp = apool.tile([128, T], f16, tag="lntmp", bufs=2,
                                         name=f"{tag}tmp{rep}_{l}_{kt}")
                        nc.vector.scalar_tensor_tensor(
                            tmp[:], xc[kt][:], sm[:, gc + kt:gc + kt + 1],
                            var[:], op0=ALU.mult, op1=ALU.mult)
                        o = apool.tile([128, T], f16, tag=otag, bufs=obufs,
                                       name=f"{tag}o{rep}_{l}_{kt}")
                        nc.scalar.activation(o[:], tmp[:], AF.Identity,
                                             bias=sm[:, bc + kt:bc + kt + 1])
                        outs.append(o)
                    return outs

                o1 = layer_norm(x16, C_G1, C_B1, "ln1", "o1", 2)

                # ---- FFN ----
                mid = []
                for m in range(NMID):
                    ps = mmslot(f"psf1{rep}_{l}_{m}")
                    for kt in range(KT):
                        for (o, w) in CH:
                            nc.tensor.matmul(
                                ps[:, o:o + w],
                                ff1_t[:, kt, m * 128:(m + 1) * 128],
                                o1[kt][:, o:o + w],
                                start=(kt == 0), stop=(kt == KT - 1))
                    mt = apool.tile([128, T], f16, tag="mid", bufs=NMID,
                                    name=f"mid{rep}_{l}_{m}")
                    nc.scalar.activation(mt[:], ps[:, 0:T], AF.Relu,
                                         bias=sm[:, C_F1B + m:C_F1B + m + 1])
                    mid.append(mt)

                x2 = []
                for d2 in range(KT):
                    psA = psmm.tile([128, 512], f32, tag="mm",
                                    name=f"psfA{rep}_{l}_{d2}")
                    psB = psmm.tile([128, 512], f32, tag="mm",
                                    name=f"psfB{rep}_{l}_{d2}")
                    psf = [psA, psB]
                    for mt in range(NMID):
                        for ci, (o, w) in enumerate(CH):
                            nc.tensor.matmul(
                                psf[ci][:, 0:w],
                                ff2_t[:, mt, d2 * 128:(d2 + 1) * 128],
                                mid[mt][:, o:o + w],
                                start=(mt == 0), stop=(mt == NMID - 1))
                    xt = apool.tile([128, T], f16, tag="x", bufs=4,
                                    name=f"x2{rep}_{l}_{d2}")
                    for ci, (o, w) in enumerate(CH):
                        nc.vector.scalar_tensor_tensor(
                            xt[:, o:o + w], psf[ci][:, 0:w],
                            sm[:, C_F2B + d2:C_F2B + d2 + 1], o1[d2][:, o:o + w],
                            op0=ALU.add, op1=ALU.add)
                    x2.append(xt)

                h16 = layer_norm(x2, C_G2, C_B2, "ln2", "h", 4)

                if l == L_RUN - 1:
                    for kt in range(KT):
                        nc.sync.dma_start(out_d[kt * 128:(kt + 1) * 128, :],
                                          h16[kt][:])

    nc.compile()
    return nc


def _fold_weights(wqkv_w, wqkv_b, A1, A2, A3, A4, tnb, out_w, out_b):
    """Fold TN contraction into dense weights; fold v-bias into out bias;
    fold 1/sqrt(D) into Q."""
    wqkv_w = np.asarray(wqkv_w, np.float32)
    wqkv_b = np.asarray(wqkv_b, np.float32)
    out_w = np.asarray(out_w, np.float32)
    out_b = np.asarray(out_b, np.float32)
    tnb = np.asarray(tnb, np.float32)
    scale = 1.0 / np.sqrt(np.float32(D))

    W_full = np.zeros((L, 3, D, H * D), np.float32)
    b_full = np.zeros((L, 3, H * D), np.float32)
    for l in range(L):
        for x in range(3):
            wt = np.einsum('pmi,qmnj,rnok,tol->pqrtijkl',
                           np.asarray(A1[l, x], np.float64),
                           np.asarray(A2[l, x], np.float64),
                           np.asarray(A3[l, x], np.float64),
                           np.asarray(A4[l, x], np.float64),
                           optimize=True).reshape(D, 4 * D).astype(np.float32)
            W_full[l, x] = np.concatenate([wqkv_w[l, x], wt], axis=1)
            b_full[l, x] = np.concatenate([wqkv_b[l, x], tnb[l, x]])
    W_full[:, 0] *= scale
    b_full[:, 0] *= scale

    wqk = np.concatenate([W_full[:, 0], W_full[:, 1]], axis=2)   # [L, 256, 3072]
    bqk = np.concatenate([b_full[:, 0], b_full[:, 1]], axis=1)   # [L, 3072]
    wv = W_full[:, 2]                                            # [L, 256, 1536]
    bv = b_full[:, 2]
    obe = out_b + np.einsum('lc,lcd->ld', bv, out_w)             # [L, 256]
    return wqk, bqk, wv, obe


def kernel(**inputs):
    tokens = np.asarray(inputs["tokens"])
    tok_emb = np.asarray(inputs["tok_emb"], np.float32)
    pos_emb = np.asarray(inputs["pos_emb"], np.float32)

    wqk, bqk, wv, obe = _fold_weights(
        inputs["wqkv_w"], inputs["wqkv_b"], inputs["A1"], inputs["A2"],
        inputs["A3"], inputs["A4"], inputs["tnb"], inputs["out_w"],
        inputs["out_b"])
    ff1 = np.asarray(inputs["ff1_w"], np.float32)
    ff2 = np.asarray(inputs["ff2_w"], np.float32)
    ow = np.asarray(inputs["out_w"], np.float32)

    # contiguous device layouts
    wqk_p = np.ascontiguousarray(
        wqk.reshape(L, KT, 128, 2 * H * D)).astype(np.float16)
    wv_p = np.ascontiguousarray(wv.reshape(L, KT, 128, H * D)).astype(np.float16)
    ow_p = np.ascontiguousarray(
        ow.reshape(L, 12, 128, D).transpose(0, 2, 1, 3)).astype(np.float16)
    ff1_p = np.ascontiguousarray(
        ff1.reshape(L, KT, 128, DFF).transpose(0, 2, 1, 3)).astype(np.float16)
    ff2_p = np.ascontiguousarray(
        ff2.reshape(L, NMID, 128, D).transpose(0, 2, 1, 3)).astype(np.float16)

    smalls = np.zeros((L, 128, 44), np.float32)
    smalls[:, :, 0:24] = bqk.reshape(L, 24, 128).transpose(0, 2, 1)
    smalls[:, :, 24:26] = obe.reshape(L, 2, 128).transpose(0, 2, 1)
    smalls[:, :, 26:34] = np.asarray(inputs["ff1_b"], np.float32).reshape(
        L, 8, 128).transpose(0, 2, 1)
    smalls[:, :, 34:36] = np.asarray(inputs["ff2_b"], np.float32).reshape(
        L, 2, 128).transpose(0, 2, 1)
    for ci, nm in ((36, "ln1_g"), (38, "ln1_b"), (40, "ln2_g"), (42, "ln2_b")):
        smalls[:, :, ci:ci + 2] = np.asarray(inputs[nm], np.float32).reshape(
            L, 2, 128).transpose(0, 2, 1)

    h0 = tok_emb[tokens] + pos_emb[None]                     # [B, S, D] f32
    maskbias = np.where(tokens == 0, np.float32(-1e9), np.float32(0.0))

    shared = {"wqk": wqk_p, "wv": wv_p, "ow": ow_p, "ff1": ff1_p,
              "ff2": ff2_p, "smalls": smalls}
    in_maps = []
    for c in range(N_CORES):
        hc = h0[c * BS:(c + 1) * BS].reshape(T, D)           # [800, 256]
        h0_dim = np.ascontiguousarray(hc.T.reshape(KT, 128, T)).astype(np.float16)
        mb = maskbias[c * BS:(c + 1) * BS]                   # [4, 200]
        maskT = np.full((128, 2 * BS), np.float32(-1e9))
        for b in range(BS):
            maskT[0:128, 2 * b] = mb[b, 0:128]
            maskT[0:S - 128, 2 * b + 1] = mb[b, 128:S]
        m = dict(shared)
        m["h0"] = h0_dim
        m["maskT"] = np.ascontiguousarray(maskT)
        in_maps.append(m)

    if "nc" not in _CACHE:
        _CACHE["nc"] = _build_program()
    nc = _CACHE["nc"]
    _CACHE["in_maps"] = in_maps

    res = run_bass_kernel_spmd(nc, in_maps, list(range(N_CORES)))
    out = np.concatenate(
        [res.results[c]["out"].astype(np.float32).T.reshape(BS, S, D)
         for c in range(N_CORES)], axis=0)
    return out


if __name__ == "__main__":
    data = np.load("/tmp/ref_data.npz")
    inputs = {k: data[k] for k in data.files if k != "expected"}
    got = kernel(**inputs)
    exp = data["expected"]
    err = np.abs(got - exp).max() / np.abs(exp).max()
    print(f"Relative error: {err:.3e}")


# revision 6
# speedup vs baseline: 1.0007x; 1.0007x over previous
"""Trainium2 Bass kernel for nn_BERT_tensor — instruction-minimized redesign.

Strategy vs baseline:
  - Data-parallel over batch: 4 seqs (800 tokens) per core x 8 cores.
  - TN (MPO) contraction folded into dense QKV weights on host.
  - Everything dim-major [256, 800]; NO PE transposes anywhere.
  - Attention computed transposed: scoresT[k,q] = K^T Q per (seq, head) with
    pad-mask as per-partition activation bias; exp in bf16 (fp32-range
    exponent, no max-subtraction needed: |scores| <= ~35); softmax
    denominator via ones-vector matmul (partition reduction on PE);
    normalization folded into the PSUM->SBUF move of ctx.
  - LayerNorm dim-major: mean/var via ones-matmul partition reductions,
    per-token stats broadcast back via rank-1 matmul; gamma/beta are
    per-partition scalars (free with ACT bias / DVE scalar ops).
  - Single fp16 h state (no separate f32 residual copy).
  - All weights host-prepacked so every DMA is contiguous; 8 DMAs/layer.
"""
import numpy as np
from contextlib import ExitStack

import concourse.bass as bass
import concourse.bacc as bacc
import concourse.tile as tile
import concourse.mybir as mybir
import concourse.bass_isa as bass_isa
from concourse.bass_utils import run_bass_kernel_spmd

dt = mybir.dt
AF = mybir.ActivationFunctionType
ALU = mybir.AluOpType

B, S, D = 32, 200, 256
H, DFF, VOCAB, L, TD = 6, 1024, 3500, 8, 2
N_CORES = 8
BS = B // N_CORES            # 4 seqs per core
T = BS * S                   # 800 tokens per core
KT = D // 128                # 2 tiles over emb dim
NQK = 24                     # m-tiles over Q|K outdim (3072)
NMID = DFF // 128            # 8 tiles over ffn hidden
CH = [(0, 512), (512, 288)]  # bank-aligned free chunks of 800
CH3 = [(0, 512), (512, 512), (1024, 176)]  # chunks of 1200
SEQT = [(0, 128), (128, 72)]
EPS = 1e-6

import os
L_RUN = int(os.environ.get("BERT_L_RUN", str(L)))
REP = int(os.environ.get("BERT_REP", "1"))

f16 = dt.float16
bf16 = dt.bfloat16
f32 = dt.float32

_CACHE = {}


def _build_program():
    nc = bacc.Bacc("TRN2", target_bir_lowering=False, debug=False,
                   num_devices=N_CORES)
    inp = {}

    def din(name, shape, dty):
        inp[name] = nc.dram_tensor(name, list(shape), dty, kind="ExternalInput").ap()
        return inp[name]

    h0_d = din("h0", [KT, 128, T], f16)
    mask_d = din("maskT", [128, 2 * BS], f32)
    wqp_d = din("wqp", [L, 128, KT, H * D], f16)
    wv_d = din("wv", [L, 128, KT, H * D], f16)
    ow_d = din("ow", [L, 128, 12, D], f16)
    ff1_d = din("ff1", [L, 128, KT, DFF], f16)
    ff2_d = din("ff2", [L, 128, NMID, D], f16)
    sm_d = din("smalls", [L, 128, 32], f32)
    out_d = nc.dram_tensor("out", [D, T], f16, kind="ExternalOutput").ap()

    # smalls column map
    C_BQK, C_OBE, C_F1B, C_F2B = 0, 12, 14, 22
    C_G1, C_B1, C_G2, C_B2 = 24, 26, 28, 30

    with tile.TileContext(nc) as tc:
        with ExitStack() as ctx:
            cpool = ctx.enter_context(tc.tile_pool(name="const", bufs=1))
            wpool = ctx.enter_context(tc.tile_pool(name="weights", bufs=1))
            apool = ctx.enter_context(tc.tile_pool(name="acts", bufs=1))
            psmm = ctx.enter_context(tc.tile_pool(name="psmm", bufs=2, space="PSUM"))

            mask_t = cpool.tile([128, 2 * BS], f32, tag="mask", name="mask_t")
            nc.sync.dma_start(mask_t[:], mask_d[:])
            eps_t = cpool.tile([128, 1], f32, tag="eps", name="eps_t")
            nc.vector.memset(eps_t[:], EPS)

            def mmslot(name, shape=(128, 1536), dty=f32):
                return psmm.tile(list(shape), dty, tag="mm", name=name)

            def arslot(name):
                return apool.tile([128, H * S], f32, tag="ar", bufs=2, name=name)

            for rep in range(REP):
              h16 = []
              for kt in range(KT):
                  t = apool.tile([128, T], f16, tag="h", bufs=4,
                                 name=f"h_init{rep}_{kt}")
                  nc.sync.dma_start(t[:], h0_d[kt])
                  h16.append(t)

              for l in range(L_RUN):
                # ---- layer weights (contiguous DMAs) ----
                wqp_t = wpool.tile([128, KT, H * D], f16, tag="wqp", bufs=2,
                                   name=f"wqp{rep}_{l}")
                nc.sync.dma_start(wqp_t[:], wqp_d[l])
                wv_t = wpool.tile([128, KT, H * D], f16, tag="wv", bufs=2,
                                  name=f"wv{rep}_{l}")
                nc.sync.dma_start(wv_t[:], wv_d[l])
                ow_t = wpool.tile([128, 12, D], f16, tag="ow", bufs=2,
                                  name=f"ow{rep}_{l}")
                nc.sync.dma_start(ow_t[:], ow_d[l])
                ff1_t = wpool.tile([128, KT, DFF], f16, tag="ff1", bufs=2,
                                   name=f"ff1{rep}_{l}")
                nc.sync.dma_start(ff1_t[:], ff1_d[l])
                ff2_t = wpool.tile([128, NMID, D], f16, tag="ff2", bufs=2,
                                   name=f"ff2{rep}_{l}")
                nc.sync.dma_start(ff2_t[:], ff2_d[l])
                sm = wpool.tile([128, 32], f32, tag="sm", bufs=2,
                                name=f"sm{rep}_{l}")
                nc.sync.dma_start(sm[:], sm_d[l])

                # ---- Q' projection (dim-major fp16 [1536, 800]);
                # K is h itself: scoresT = h^T (M h + u) with M = Wk Wq^T
                qk = []
                for m in range(12):
                    ps = mmslot(f"psqk{rep}_{l}_{m}")
                    for kt in range(KT):
                        for (o, w) in CH:
                            nc.tensor.matmul(
                                ps[:, o:o + w],
                                wqp_t[:, kt, m * 128:(m + 1) * 128],
                                h16[kt][:, o:o + w],
                                start=(kt == 0), stop=(kt == KT - 1))
                    qt = apool.tile([128, T], f16, tag="qk", bufs=12,
                                    name=f"qk{rep}_{l}_{m}")
                    nc.scalar.activation(qt[:], ps[:, 0:T], AF.Identity,
                                         bias=sm[:, C_BQK + m:C_BQK + m + 1])
                    qk.append(qt)

                # ---- attention per sequence ----
                ctxA = apool.tile([128, 12, T], f16, tag="ctx", bufs=1,
                                  name=f"ctx{rep}_{l}")
                for b in range(BS):
                    # V token-major bf16 [ts, 1536] per seq-tile
                    vt = []
                    for ti, (to, ts) in enumerate(SEQT):
                        ps = mmslot(f"psv{rep}_{l}_{b}_{ti}")
                        for kt in range(KT):
                            for c in range(3):
                                nc.tensor.matmul(
                                    ps[0:ts, c * 512:(c + 1) * 512],
                                    h16[kt][:, b * S + to:b * S + to + ts],
                                    wv_t[:, kt, c * 512:(c + 1) * 512],
                                    start=(kt == 0), stop=(kt == KT - 1))
                        v = apool.tile([128, H * D], bf16, tag="v", bufs=4,
                                       name=f"v{rep}_{l}_{b}_{ti}")
                        nc.scalar.activation(v[0:ts, :], ps[0:ts, 0:H * D], AF.Copy)
                        vt.append(v)

                    # scoresT + exp (bf16), head-packed PSUM [128, 6, 256]
                    expT = []
                    for ti, (ko, ks) in enumerate(SEQT):
                        ps = psmm.tile([128, H, 256], f32, tag="mm",
                                       name=f"pssc{rep}_{l}_{b}_{ti}")
                        for h in range(H):
                            for kt in range(KT):
                                nc.tensor.matmul(
                                    ps[0:ks, h, 0:S],
                                    h16[kt][:, b * S + ko:b * S + ko + ks],
                                    qk[h * KT + kt][:, b * S:(b + 1) * S],
                                    start=(kt == 0), stop=(kt == KT - 1))
                        e = apool.tile([128, H, S], bf16, tag="expT", bufs=4,
                                       name=f"expT{rep}_{l}_{b}_{ti}")
                        nc.scalar.activation(
                            e[0:ks, :, :], ps[0:ks, :, 0:S], AF.Exp,
                            bias=mask_t[0:ks, b * 2 + ti:b * 2 + ti + 1])
                        expT.append(e)

                    # softmax denominators via gpsimd partition reductions
                    ar0 = arslot(f"ar0{rep}_{l}_{b}")
                    ar1 = arslot(f"ar1{rep}_{l}_{b}")
                    nc.gpsimd.partition_all_reduce(
                        ar0[:, :], expT[0][:, :, :], channels=128,
                        reduce_op=bass_isa.ReduceOp.add)
                    nc.gpsimd.partition_all_reduce(
                        ar1[0:72, :], expT[1][0:72, :, :], channels=72,
                        reduce_op=bass_isa.ReduceOp.add)
                    rs = apool.tile([1, H * S], f32, tag="rs", bufs=1,
                                    name=f"rs{rep}_{l}_{b}")
                    nc.vector.tensor_tensor(rs[0:1, :], ar0[0:1, :], ar1[0:1, :],
                                            op=ALU.add)
                    nc.vector.reciprocal(rs[0:1, :], rs[0:1, :])
                    rsb = apool.tile([128, H * S], f32, tag="rsb", bufs=1,
                                     name=f"rsb{rep}_{l}_{b}")
                    nc.gpsimd.partition_broadcast(rsb[:, :], rs[0:1, :])

                    # ctx dim-major, packed [128, 6, 256] per 3-head half;
                    # softmax normalization fused into the PSUM->SBUF move
                    for half in range(2):
                        pc = psmm.tile([128, 6, 256], f32, tag="mm",
                                       name=f"psctx{rep}_{l}_{b}_{half}")
                        for j in range(6):
                            h, dtile = half * 3 + j // 2, j % 2
                            for ti, (ko, ks) in enumerate(SEQT):
                                nc.tensor.matmul(
                                    pc[:, j, 0:S],
                                    vt[ti][0:ks, h * D + dtile * 128:
                                           h * D + (dtile + 1) * 128],
                                    expT[ti][0:ks, h, :],
                                    start=(ti == 0), stop=(ti == 1))
                        for dtile in range(2):
                            nc.vector.tensor_tensor(
                                ctxA[:, half * 6 + dtile:half * 6 + 6:2,
                                     b * S:(b + 1) * S],
                                pc[:, dtile:6:2, 0:S],
                                rsb[:, half * 3 * S:(half + 1) * 3 * S],
                                op=ALU.mult)

                # ---- out projection + residual -> x16 ----
                x16 = []
                for d2 in range(KT):
                    ps = mmslot(f"pso{rep}_{l}_{d2}")
                    for ct in range(12):
                        for (o, w) in CH:
                            nc.tensor.matmul(
                                ps[:, o:o + w],
                                ow_t[:, ct, d2 * 128:(d2 + 1) * 128],
                                ctxA[:, ct, o:o + w],
                                start=(ct == 0), stop=(ct == 11))
                    xt = apool.tile([128, T], f16, tag="x", bufs=4,
                                    name=f"x{rep}_{l}_{d2}")
                    nc.vector.scalar_tensor_tensor(
                        xt[:], ps[:, 0:T],
                        sm[:, C_OBE + d2:C_OBE + d2 + 1], h16[d2][:],
                        op0=ALU.add, op1=ALU.add)
                    x16.append(xt)

                def layer_norm(xt, gc, bc, tag, otag, obufs):
                    """dim-major LN: xt 2 fp16 [128,800] tiles -> 2 fp16 tiles."""
                    # -mean
                    # channel sums of x via gpsimd all-reduce (broadcast out)
                    arm = []
                    for kt in range(KT):
                        a = arslot(f"{tag}arm{rep}_{l}_{kt}")
                        nc.gpsimd.partition_all_reduce(
                            a[:, 0:T], xt[kt][:, :], channels=128,
                            reduce_op=bass_isa.ReduceOp.add)
                        arm.append(a)
                    xc, sq = [], []
                    for kt in range(KT):
                        t1 = apool.tile([128, T], f16, tag="lnxc", bufs=4,
                                        name=f"{tag}t1{rep}_{l}_{kt}")
                        nc.vector.scalar_tensor_tensor(
                            t1[:], arm[0][:, 0:T], -1.0 / D, xt[kt][:],
                            op0=ALU.mult, op1=ALU.add)
                        c = apool.tile([128, T], f16, tag="lnxc", bufs=4,
                                       name=f"{tag}xc{rep}_{l}_{kt}")
                        nc.vector.scalar_tensor_tensor(
                            c[:], arm[1][:, 0:T], -1.0 / D, t1[:],
                            op0=ALU.mult, op1=ALU.add)
                        xc.append(c)
                        s = apool.tile([128, T], f16, tag="lnsq", bufs=2,
                                       name=f"{tag}sq{rep}_{l}_{kt}")
                        nc.scalar.activation(s[:], c[:], AF.Square)
                        sq.append(s)
                    # rstd = 1/sqrt(var+eps), broadcast across partitions
                    arv = []
                    for kt in range(KT):
                        a = arslot(f"{tag}arv{rep}_{l}_{kt}")
                        nc.gpsimd.partition_all_reduce(
                            a[:, 0:T], sq[kt][:, :], channels=128,
                            reduce_op=bass_isa.ReduceOp.add)
                        arv.append(a)
                    var = apool.tile([128, T], f32, tag="lnvar", bufs=1,
                                     name=f"{tag}var{rep}_{l}")
                    nc.vector.tensor_tensor(var[:], arv[0][:, 0:T], arv[1][:, 0:T],
                                            op=ALU.add)
                    nc.scalar.activation(var[:], var[:], AF.Sqrt, scale=1.0 / D,
                                         bias=eps_t[:, :])
                    nc.vector.reciprocal(var[:], var[:])
                    outs = []
                    for kt in range(KT):
                        tmp = apool.tile([128, T], f16, tag="lntmp", bufs=2,
                                         name=f"{tag}tmp{rep}_{l}_{kt}")
                        nc.vector.scalar_tensor_tensor(
                            tmp[:], xc[kt][:], sm[:, gc + kt:gc + kt + 1],
                            var[:], op0=ALU.mult, op1=ALU.mult)
                        o = apool.tile([128, T], f16, tag=otag, bufs=obufs,
                                       name=f"{tag}o{rep}_{l}_{kt}")
                        nc.scalar.activation(o[:], tmp[:], AF.Identity,
                                             bias=sm[:, bc + kt:bc + kt + 1])
                        outs.append(o)
                    return outs

                o1 = layer_norm(x16, C_G1, C_B1, "ln1", "o1", 2)

                # ---- FFN ----
                mid = []
                for m in range(NMID):
                    ps = mmslot(f"psf1{rep}_{l}_{m}")
                    for kt in range(KT):
                        for (o, w) in CH:
                            nc.tensor.matmul(
                                ps[:, o:o + w],
                                ff1_t[:, kt, m * 128:(m + 1) * 128],
                                o1[kt][:, o:o + w],
                                start=(kt == 0), stop=(kt == KT - 1))
                    mt = apool.tile([128, T], f16, tag="mid", bufs=NMID,
                                    name=f"mid{rep}_{l}_{m}")
                    nc.scalar.activation(mt[:], ps[:, 0:T], AF.Relu,
                                         bias=sm[:, C_F1B + m:C_F1B + m + 1])
                    mid.append(mt)

                x2 = []
                for d2 in range(KT):
                    ps = mmslot(f"psf2{rep}_{l}_{d2}")
                    for mt in range(NMID):
                        for (o, w) in CH:
                            nc.tensor.matmul(
                                ps[:, o:o + w],
                                ff2_t[:, mt, d2 * 128:(d2 + 1) * 128],
                                mid[mt][:, o:o + w],
                                start=(mt == 0), stop=(mt == NMID - 1))
                    xt = apool.tile([128, T], f16, tag="x", bufs=4,
                                    name=f"x2{rep}_{l}_{d2}")
                    nc.vector.scalar_tensor_tensor(
                        xt[:], ps[:, 0:T],
                        sm[:, C_F2B + d2:C_F2B + d2 + 1], o1[d2][:],
                        op0=ALU.add, op1=ALU.add)
                    x2.append(xt)

                h16 = layer_norm(x2, C_G2, C_B2, "ln2", "h", 4)

                if l == L_RUN - 1:
                    for kt in range(KT):
                        nc.sync.dma_start(out_d[kt * 128:(kt + 1) * 128, :],
                                          h16[kt][:])

    nc.compile()
    return nc


def _fold_weights(wqkv_w, wqkv_b, A1, A2, A3, A4, tnb, out_w, out_b):
    """Fold TN contraction into dense weights; fold v-bias into out bias;
    fold 1/sqrt(D) into Q."""
    wqkv_w = np.asarray(wqkv_w, np.float32)
    wqkv_b = np.asarray(wqkv_b, np.float32)
    out_w = np.asarray(out_w, np.float32)
    out_b = np.asarray(out_b, np.float32)
    tnb = np.asarray(tnb, np.float32)
    scale = 1.0 / np.sqrt(np.float32(D))

    W_full = np.zeros((L, 3, D, H * D), np.float32)
    b_full = np.zeros((L, 3, H * D), np.float32)
    for l in range(L):
        for x in range(3):
            wt = np.einsum('pmi,qmnj,rnok,tol->pqrtijkl',
                           np.asarray(A1[l, x], np.float64),
                           np.asarray(A2[l, x], np.float64),
                           np.asarray(A3[l, x], np.float64),
                           np.asarray(A4[l, x], np.float64),
                           optimize=True).reshape(D, 4 * D).astype(np.float32)
            W_full[l, x] = np.concatenate([wqkv_w[l, x], wt], axis=1)
            b_full[l, x] = np.concatenate([wqkv_b[l, x], tnb[l, x]])
    W_full[:, 0] *= scale
    b_full[:, 0] *= scale

    # fold K into Q: scoresT[k,q] = h_k . (M h_q + u) with
    # M = Wk Wq^T (per head), u = Wk bq; per-q softmax constants dropped
    Mq = np.zeros((L, H * D, D), np.float32)     # [L, 1536(out), 256(emb)]
    u = np.zeros((L, H * D), np.float32)
    for l in range(L):
        for h in range(H):
            s = slice(h * D, (h + 1) * D)
            wq = W_full[l, 0][:, s].astype(np.float64)   # [256e, 256j]
            wk = W_full[l, 1][:, s].astype(np.float64)
            Mq[l, s, :] = (wk @ wq.T).astype(np.float32)  # [256d, 256e]
            u[l, s] = (wk @ b_full[l, 0][s].astype(np.float64)).astype(np.float32)
    wv = W_full[:, 2]                                            # [L, 256, 1536]
    bv = b_full[:, 2]
    obe = out_b + np.einsum('lc,lcd->ld', bv, out_w)             # [L, 256]
    return Mq, u, wv, obe


def kernel(**inputs):
    tokens = np.asarray(inputs["tokens"])
    tok_emb = np.asarray(inputs["tok_emb"], np.float32)
    pos_emb = np.asarray(inputs["pos_emb"], np.float32)

    Mq, uq, wv, obe = _fold_weights(
        inputs["wqkv_w"], inputs["wqkv_b"], inputs["A1"], inputs["A2"],
        inputs["A3"], inputs["A4"], inputs["tnb"], inputs["out_w"],
        inputs["out_b"])
    ff1 = np.asarray(inputs["ff1_w"], np.float32)
    ff2 = np.asarray(inputs["ff2_w"], np.float32)
    ow = np.asarray(inputs["out_w"], np.float32)

    # contiguous device layouts
    # wqp[l, p, kt, o] = Mq[l, o, kt*128 + p]
    wqp_p = np.ascontiguousarray(Mq.transpose(0, 2, 1).reshape(
        L, KT, 128, H * D).transpose(0, 2, 1, 3)).astype(np.float16)
    wv_p = np.ascontiguousarray(wv.reshape(L, KT, 128, H * D).transpose(
        0, 2, 1, 3)).astype(np.float16)
    ow_p = np.ascontiguousarray(
        ow.reshape(L, 12, 128, D).transpose(0, 2, 1, 3)).astype(np.float16)
    ff1_p = np.ascontiguousarray(
        ff1.reshape(L, KT, 128, DFF).transpose(0, 2, 1, 3)).astype(np.float16)
    ff2_p = np.ascontiguousarray(
        ff2.reshape(L, NMID, 128, D).transpose(0, 2, 1, 3)).astype(np.float16)

    smalls = np.zeros((L, 128, 32), np.float32)
    smalls[:, :, 0:12] = uq.reshape(L, 12, 128).transpose(0, 2, 1)
    smalls[:, :, 12:14] = obe.reshape(L, 2, 128).transpose(0, 2, 1)
    smalls[:, :, 14:22] = np.asarray(inputs["ff1_b"], np.float32).reshape(
        L, 8, 128).transpose(0, 2, 1)
    smalls[:, :, 22:24] = np.asarray(inputs["ff2_b"], np.float32).reshape(
        L, 2, 128).transpose(0, 2, 1)
    for ci, nm in ((24, "ln1_g"), (26, "ln1_b"), (28, "ln2_g"), (30, "ln2_b")):
        smalls[:, :, ci:ci + 2] = np.asarray(inputs[nm], np.float32).reshape(
            L, 2, 128).transpose(0, 2, 1)

    h0 = tok_emb[tokens] + pos_emb[None]                     # [B, S, D] f32
    maskbias = np.where(tokens == 0, np.float32(-1e9), np.float32(0.0))

    shared = {"wqp": wqp_p, "wv": wv_p, "ow": ow_p, "ff1": ff1_p,
              "ff2": ff2_p, "smalls": smalls}
    in_maps = []
    for c in range(N_CORES):
        hc = h0[c * BS:(c + 1) * BS].reshape(T, D)           # [800, 256]
        h0_dim = np.ascontiguousarray(hc.T.reshape(KT, 128, T)).astype(np.float16)
        mb = maskbias[c * BS:(c + 1) * BS]                   # [4, 200]
        maskT = np.full((128, 2 * BS), np.float32(-1e9))
        for b in range(BS):
            maskT[0:128, 2 * b] = mb[b, 0:128]
            maskT[0:S - 128, 2 * b + 1] = mb[b, 128:S]
        m = dict(shared)
        m["h0"] = h0_dim
        m["maskT"] = np.ascontiguousarray(maskT)
        in_maps.append(m)

    if "nc" not in _CACHE:
        _CACHE["nc"] = _build_program()
    nc = _CACHE["nc"]
    _CACHE["in_maps"] = in_maps

    res = run_bass_kernel_spmd(nc, in_maps, list(range(N_CORES)))
    out = np.concatenate(
        [res.results[c]["out"].astype(np.float32).T.reshape(BS, S, D)
         for c in range(N_CORES)], axis=0)
    return out


if __name__ == "__main__":
    data = np.load("/tmp/ref_data.npz")
    inputs = {k: data[k] for k in data.files if k != "expected"}
    got = kernel(**inputs)
    exp = data["expected"]
    err = np.abs(got - exp).max() / np.abs(exp).max()
    print(f"Relative error: {err:.3e}")


# revision 7
# speedup vs baseline: 1.0023x; 1.0016x over previous
"""Trainium2 Bass kernel for nn_BERT_tensor — instruction-minimized redesign.

Strategy vs baseline:
  - Data-parallel over batch: 4 seqs (800 tokens) per core x 8 cores.
  - TN (MPO) contraction folded into dense QKV weights on host.
  - Everything dim-major [256, 800]; NO PE transposes anywhere.
  - Attention computed transposed: scoresT[k,q] = K^T Q per (seq, head) with
    pad-mask as per-partition activation bias; exp in bf16 (fp32-range
    exponent, no max-subtraction needed: |scores| <= ~35); softmax
    denominator via ones-vector matmul (partition reduction on PE);
    normalization folded into the PSUM->SBUF move of ctx.
  - LayerNorm dim-major: mean/var via ones-matmul partition reductions,
    per-token stats broadcast back via rank-1 matmul; gamma/beta are
    per-partition scalars (free with ACT bias / DVE scalar ops).
  - Single fp16 h state (no separate f32 residual copy).
  - All weights host-prepacked so every DMA is contiguous; 8 DMAs/layer.
"""
import numpy as np
from contextlib import ExitStack

import concourse.bass as bass
import concourse.bacc as bacc
import concourse.tile as tile
import concourse.mybir as mybir
import concourse.bass_isa as bass_isa
from concourse.bass_utils import run_bass_kernel_spmd

dt = mybir.dt
AF = mybir.ActivationFunctionType
ALU = mybir.AluOpType

B, S, D = 32, 200, 256
H, DFF, VOCAB, L, TD = 6, 1024, 3500, 8, 2
N_CORES = 8
BS = B // N_CORES            # 4 seqs per core
T = BS * S                   # 800 tokens per core
KT = D // 128                # 2 tiles over emb dim
NQK = 24                     # m-tiles over Q|K outdim (3072)
NMID = DFF // 128            # 8 tiles over ffn hidden
CH = [(0, 512), (512, 288)]  # bank-aligned free chunks of 800
CH3 = [(0, 512), (512, 512), (1024, 176)]  # chunks of 1200
SEQT = [(0, 128), (128, 72)]
EPS = 1e-6

import os
L_RUN = int(os.environ.get("BERT_L_RUN", str(L)))
REP = int(os.environ.get("BERT_REP", "1"))

f16 = dt.float16
bf16 = dt.bfloat16
f32 = dt.float32

_CACHE = {}


def _build_program():
    nc = bacc.Bacc("TRN2", target_bir_lowering=False, debug=False,
                   num_devices=N_CORES)
    inp = {}

    def din(name, shape, dty):
        inp[name] = nc.dram_tensor(name, list(shape), dty, kind="ExternalInput").ap()
        return inp[name]

    h0_d = din("h0", [KT, 128, T], f16)
    mask_d = din("maskT", [128, 2 * BS], f32)
    wqp_d = din("wqp", [L, 128, KT, H * D], f16)
    wv_d = din("wv", [L, 128, KT, H * D], f16)
    ow_d = din("ow", [L, 128, 12, D], f16)
    ff1_d = din("ff1", [L, 128, KT, DFF], f16)
    ff2_d = din("ff2", [L, 128, NMID, D], f16)
    sm_d = din("smalls", [L, 128, 32], f32)
    out_d = nc.dram_tensor("out", [D, T], f16, kind="ExternalOutput").ap()

    # smalls column map
    C_BQK, C_OBE, C_F1B, C_F2B = 0, 12, 14, 22
    C_G1, C_B1, C_G2, C_B2 = 24, 26, 28, 30

    with tile.TileContext(nc) as tc:
        with ExitStack() as ctx:
            cpool = ctx.enter_context(tc.tile_pool(name="const", bufs=1))
            wpool = ctx.enter_context(tc.tile_pool(name="weights", bufs=1))
            apool = ctx.enter_context(tc.tile_pool(name="acts", bufs=1))
            psmm = ctx.enter_context(tc.tile_pool(name="psmm", bufs=2, space="PSUM"))

            mask_t = cpool.tile([128, 2 * BS], f32, tag="mask", name="mask_t")
            nc.sync.dma_start(mask_t[:], mask_d[:])
            eps_t = cpool.tile([128, 1], f32, tag="eps", name="eps_t")
            nc.vector.memset(eps_t[:], EPS)

            def mmslot(name, shape=(128, 1536), dty=f32):
                return psmm.tile(list(shape), dty, tag="mm", name=name)

            def arslot(name):
                return apool.tile([128, H * S], f32, tag="ar", bufs=2, name=name)

            for rep in range(REP):
              h16 = []
              for kt in range(KT):
                  t = apool.tile([128, T], f16, tag="h", bufs=4,
                                 name=f"h_init{rep}_{kt}")
                  nc.sync.dma_start(t[:], h0_d[kt])
                  h16.append(t)

              for l in range(L_RUN):
                # ---- layer weights (contiguous DMAs) ----
                wqp_t = wpool.tile([128, KT, H * D], f16, tag="wqp", bufs=2,
                                   name=f"wqp{rep}_{l}")
                nc.sync.dma_start(wqp_t[:], wqp_d[l])
                wv_t = wpool.tile([128, KT, H * D], f16, tag="wv", bufs=2,
                                  name=f"wv{rep}_{l}")
                nc.sync.dma_start(wv_t[:], wv_d[l])
                ow_t = wpool.tile([128, 12, D], f16, tag="ow", bufs=2,
                                  name=f"ow{rep}_{l}")
                nc.sync.dma_start(ow_t[:], ow_d[l])
                ff1_t = wpool.tile([128, KT, DFF], f16, tag="ff1", bufs=2,
                                   name=f"ff1{rep}_{l}")
                nc.sync.dma_start(ff1_t[:], ff1_d[l])
                ff2_t = wpool.tile([128, NMID, D], f16, tag="ff2", bufs=2,
                                   name=f"ff2{rep}_{l}")
                nc.sync.dma_start(ff2_t[:], ff2_d[l])
                sm = wpool.tile([128, 32], f32, tag="sm", bufs=2,
                                name=f"sm{rep}_{l}")
                nc.sync.dma_start(sm[:], sm_d[l])

                # ---- Q' projection (dim-major fp16 [1536, 800]);
                # K is h itself: scoresT = h^T (M h + u) with M = Wk Wq^T
                qk = [apool.tile([128, H, T], f16, tag=f"qk{kt}", bufs=1,
                                 name=f"qk{rep}_{l}_{kt}") for kt in range(KT)]
                for m in range(12):
                    h, kto = m // KT, m % KT
                    ps = mmslot(f"psqk{rep}_{l}_{m}")
                    for kt in range(KT):
                        for (o, w) in CH:
                            nc.tensor.matmul(
                                ps[:, o:o + w],
                                wqp_t[:, kt, m * 128:(m + 1) * 128],
                                h16[kt][:, o:o + w],
                                start=(kt == 0), stop=(kt == KT - 1))
                    nc.scalar.activation(qk[kto][:, h, :], ps[:, 0:T],
                                         AF.Identity,
                                         bias=sm[:, C_BQK + m:C_BQK + m + 1])

                # ---- attention per sequence ----
                ctxA = apool.tile([128, 12, T], f16, tag="ctx", bufs=1,
                                  name=f"ctx{rep}_{l}")
                for b in range(BS):
                    # V token-major bf16 [ts, 1536] per seq-tile
                    vt = []
                    for ti, (to, ts) in enumerate(SEQT):
                        ps = mmslot(f"psv{rep}_{l}_{b}_{ti}")
                        for kt in range(KT):
                            for c in range(3):
                                nc.tensor.matmul(
                                    ps[0:ts, c * 512:(c + 1) * 512],
                                    h16[kt][:, b * S + to:b * S + to + ts],
                                    wv_t[:, kt, c * 512:(c + 1) * 512],
                                    start=(kt == 0), stop=(kt == KT - 1))
                        v = apool.tile([128, H * D], bf16, tag="v", bufs=4,
                                       name=f"v{rep}_{l}_{b}_{ti}")
                        nc.scalar.activation(v[0:ts, :], ps[0:ts, 0:H * D], AF.Copy)
                        vt.append(v)

                    # scoresT + exp (bf16), head-packed PSUM [128, 6, 256]
                    expT = []
                    for ti, (ko, ks) in enumerate(SEQT):
                        ps = psmm.tile([128, H, 256], f32, tag="mm",
                                       name=f"pssc{rep}_{l}_{b}_{ti}")
                        for kt in range(KT):
                            for hp in range(3):
                                nc.tensor.matmul(
                                    ps[0:ks, 2 * hp:2 * hp + 2, 0:S],
                                    h16[kt][:, b * S + ko:b * S + ko + ks],
                                    qk[kt][:, 2 * hp:2 * hp + 2,
                                           b * S:(b + 1) * S],
                                    start=(kt == 0), stop=(kt == KT - 1))
                        e = apool.tile([128, H, S], bf16, tag="expT", bufs=4,
                                       name=f"expT{rep}_{l}_{b}_{ti}")
                        nc.scalar.activation(
                            e[0:ks, :, :], ps[0:ks, :, 0:S], AF.Exp,
                            bias=mask_t[0:ks, b * 2 + ti:b * 2 + ti + 1])
                        expT.append(e)

                    # softmax denominators via gpsimd partition reductions
                    ar0 = arslot(f"ar0{rep}_{l}_{b}")
                    ar1 = arslot(f"ar1{rep}_{l}_{b}")
                    nc.gpsimd.partition_all_reduce(
                        ar0[:, :], expT[0][:, :, :], channels=128,
                        reduce_op=bass_isa.ReduceOp.add)
                    nc.gpsimd.partition_all_reduce(
                        ar1[0:72, :], expT[1][0:72, :, :], channels=72,
                        reduce_op=bass_isa.ReduceOp.add)
                    rs = apool.tile([1, H * S], f32, tag="rs", bufs=1,
                                    name=f"rs{rep}_{l}_{b}")
                    nc.vector.tensor_tensor(rs[0:1, :], ar0[0:1, :], ar1[0:1, :],
                                            op=ALU.add)
                    nc.vector.reciprocal(rs[0:1, :], rs[0:1, :])
                    rsb = apool.tile([128, H * S], f32, tag="rsb", bufs=1,
                                     name=f"rsb{rep}_{l}_{b}")
                    nc.gpsimd.partition_broadcast(rsb[:, :], rs[0:1, :])

                    # ctx dim-major, packed [128, 6, 256] per 3-head half;
                    # softmax normalization fused into the PSUM->SBUF move
                    for half in range(2):
                        pc = psmm.tile([128, 6, 256], f32, tag="mm",
                                       name=f"psctx{rep}_{l}_{b}_{half}")
                        for j in range(6):
                            h, dtile = half * 3 + j // 2, j % 2
                            for ti, (ko, ks) in enumerate(SEQT):
                                nc.tensor.matmul(
                                    pc[:, j, 0:S],
                                    vt[ti][0:ks, h * D + dtile * 128:
                                           h * D + (dtile + 1) * 128],
                                    expT[ti][0:ks, h, :],
                                    start=(ti == 0), stop=(ti == 1))
                        for dtile in range(2):
                            nc.vector.tensor_tensor(
                                ctxA[:, half * 6 + dtile:half * 6 + 6:2,
                                     b * S:(b + 1) * S],
                                pc[:, dtile:6:2, 0:S],
                                rsb[:, half * 3 * S:(half + 1) * 3 * S],
                                op=ALU.mult)

                # ---- out projection + residual -> x16 ----
                x16 = []
                for d2 in range(KT):
                    ps = mmslot(f"pso{rep}_{l}_{d2}")
                    for ct in range(12):
                        for (o, w) in CH:
                            nc.tensor.matmul(
                                ps[:, o:o + w],
                                ow_t[:, ct, d2 * 128:(d2 + 1) * 128],
                                ctxA[:, ct, o:o + w],
                                start=(ct == 0), stop=(ct == 11))
                    xt = apool.tile([128, T], f16, tag="x", bufs=4,
                                    name=f"x{rep}_{l}_{d2}")
                    nc.vector.scalar_tensor_tensor(
                        xt[:], ps[:, 0:T],
                        sm[:, C_OBE + d2:C_OBE + d2 + 1], h16[d2][:],
                        op0=ALU.add, op1=ALU.add)
                    x16.append(xt)

                def layer_norm(xt, gc, bc, tag, otag, obufs):
                    """dim-major LN: xt 2 fp16 [128,800] tiles -> 2 fp16 tiles."""
                    # -mean
                    # channel sums of x via gpsimd all-reduce (broadcast out)
                    arm = []
                    for kt in range(KT):
                        a = arslot(f"{tag}arm{rep}_{l}_{kt}")
                        nc.gpsimd.partition_all_reduce(
                            a[:, 0:T], xt[kt][:, :], channels=128,
                            reduce_op=bass_isa.ReduceOp.add)
                        arm.append(a)
                    xc, sq = [], []
                    for kt in range(KT):
                        t1 = apool.tile([128, T], f16, tag="lnxc", bufs=4,
                                        name=f"{tag}t1{rep}_{l}_{kt}")
                        nc.vector.scalar_tensor_tensor(
                            t1[:], arm[0][:, 0:T], -1.0 / D, xt[kt][:],
                            op0=ALU.mult, op1=ALU.add)
                        c = apool.tile([128, T], f16, tag="lnxc", bufs=4,
                                       name=f"{tag}xc{rep}_{l}_{kt}")
                        nc.vector.scalar_tensor_tensor(
                            c[:], arm[1][:, 0:T], -1.0 / D, t1[:],
                            op0=ALU.mult, op1=ALU.add)
                        xc.append(c)
                        s = apool.tile([128, T], f16, tag="lnsq", bufs=2,
                                       name=f"{tag}sq{rep}_{l}_{kt}")
                        nc.scalar.activation(s[:], c[:], AF.Square)
                        sq.append(s)
                    # rstd = 1/sqrt(var+eps), broadcast across partitions
                    arv = []
                    for kt in range(KT):
                        a = arslot(f"{tag}arv{rep}_{l}_{kt}")
                        nc.gpsimd.partition_all_reduce(
                            a[:, 0:T], sq[kt][:, :], channels=128,
                            reduce_op=bass_isa.ReduceOp.add)
                        arv.append(a)
                    var = apool.tile([128, T], f32, tag="lnvar", bufs=1,
                                     name=f"{tag}var{rep}_{l}")
                    nc.vector.tensor_tensor(var[:], arv[0][:, 0:T], arv[1][:, 0:T],
                                            op=ALU.add)
                    nc.scalar.activation(var[:], var[:], AF.Sqrt, scale=1.0 / D,
                                         bias=eps_t[:, :])
                    nc.vector.reciprocal(var[:], var[:])
                    outs = []
                    for kt in range(KT):
                        tmp = apool.tile([128, T], f16, tag="lntmp", bufs=2,
                                         name=f"{tag}tmp{rep}_{l}_{kt}")
                        nc.vector.scalar_tensor_tensor(
                            tmp[:], xc[kt][:], sm[:, gc + kt:gc + kt + 1],
                            var[:], op0=ALU.mult, op1=ALU.mult)
                        o = apool.tile([128, T], f16, tag=otag, bufs=obufs,
                                       name=f"{tag}o{rep}_{l}_{kt}")
                        nc.scalar.activation(o[:], tmp[:], AF.Identity,
                                             bias=sm[:, bc + kt:bc + kt + 1])
                        outs.append(o)
                    return outs

                o1 = layer_norm(x16, C_G1, C_B1, "ln1", "o1", 2)

                # ---- FFN ----
                mid = []
                for m in range(NMID):
                    ps = mmslot(f"psf1{rep}_{l}_{m}")
                    for kt in range(KT):
                        for (o, w) in CH:
                            nc.tensor.matmul(
                                ps[:, o:o + w],
                                ff1_t[:, kt, m * 128:(m + 1) * 128],
                                o1[kt][:, o:o + w],
                                start=(kt == 0), stop=(kt == KT - 1))
                    mt = apool.tile([128, T], f16, tag="mid", bufs=NMID,
                                    name=f"mid{rep}_{l}_{m}")
                    nc.scalar.activation(mt[:], ps[:, 0:T], AF.Relu,
                                         bias=sm[:, C_F1B + m:C_F1B + m + 1])
                    mid.append(mt)

                x2 = []
                for d2 in range(KT):
                    ps = mmslot(f"psf2{rep}_{l}_{d2}")
                    for mt in range(NMID):
                        for (o, w) in CH:
                            nc.tensor.matmul(
                                ps[:, o:o + w],
                                ff2_t[:, mt, d2 * 128:(d2 + 1) * 128],
                                mid[mt][:, o:o + w],
                                start=(mt == 0), stop=(mt == NMID - 1))
                    xt = apool.tile([128, T], f16, tag="x", bufs=4,
                                    name=f"x2{rep}_{l}_{d2}")
                    nc.vector.scalar_tensor_tensor(
                        xt[:], ps[:, 0:T],
                        sm[:, C_F2B + d2:C_F2B + d2 + 1], o1[d2][:],
                        op0=ALU.add, op1=ALU.add)
                    x2.append(xt)

                h16 = layer_norm(x2, C_G2, C_B2, "ln2", "h", 4)

                if l == L_RUN - 1:
                    for kt in range(KT):
                        nc.sync.dma_start(out_d[kt * 128:(kt + 1) * 128, :],
                                          h16[kt][:])

    nc.compile()
    return nc


def _fold_weights(wqkv_w, wqkv_b, A1, A2, A3, A4, tnb, out_w, out_b):
    """Fold TN contraction into dense weights; fold v-bias into out bias;
    fold 1/sqrt(D) into Q."""
    wqkv_w = np.asarray(wqkv_w, np.float32)
    wqkv_b = np.asarray(wqkv_b, np.float32)
    out_w = np.asarray(out_w, np.float32)
    out_b = np.asarray(out_b, np.float32)
    tnb = np.asarray(tnb, np.float32)
    scale = 1.0 / np.sqrt(np.float32(D))

    W_full = np.zeros((L, 3, D, H * D), np.float32)
    b_full = np.zeros((L, 3, H * D), np.float32)
    for l in range(L):
        for x in range(3):
            wt = np.einsum('pmi,qmnj,rnok,tol->pqrtijkl',
                           np.asarray(A1[l, x], np.float64),
                           np.asarray(A2[l, x], np.float64),
                           np.asarray(A3[l, x], np.float64),
                           np.asarray(A4[l, x], np.float64),
                           optimize=True).reshape(D, 4 * D).astype(np.float32)
            W_full[l, x] = np.concatenate([wqkv_w[l, x], wt], axis=1)
            b_full[l, x] = np.concatenate([wqkv_b[l, x], tnb[l, x]])
    W_full[:, 0] *= scale
    b_full[:, 0] *= scale

    # fold K into Q: scoresT[k,q] = h_k . (M h_q + u) with
    # M = Wk Wq^T (per head), u = Wk bq; per-q softmax constants dropped
    Mq = np.zeros((L, H * D, D), np.float32)     # [L, 1536(out), 256(emb)]
    u = np.zeros((L, H * D), np.float32)
    for l in range(L):
        for h in range(H):
            s = slice(h * D, (h + 1) * D)
            wq = W_full[l, 0][:, s].astype(np.float64)   # [256e, 256j]
            wk = W_full[l, 1][:, s].astype(np.float64)
            Mq[l, s, :] = (wk @ wq.T).astype(np.float32)  # [256d, 256e]
            u[l, s] = (wk @ b_full[l, 0][s].astype(np.float64)).astype(np.float32)
    wv = W_full[:, 2]                                            # [L, 256, 1536]
    bv = b_full[:, 2]
    obe = out_b + np.einsum('lc,lcd->ld', bv, out_w)             # [L, 256]
    return Mq, u, wv, obe


def kernel(**inputs):
    tokens = np.asarray(inputs["tokens"])
    tok_emb = np.asarray(inputs["tok_emb"], np.float32)
    pos_emb = np.asarray(inputs["pos_emb"], np.float32)

    Mq, uq, wv, obe = _fold_weights(
        inputs["wqkv_w"], inputs["wqkv_b"], inputs["A1"], inputs["A2"],
        inputs["A3"], inputs["A4"], inputs["tnb"], inputs["out_w"],
        inputs["out_b"])
    ff1 = np.asarray(inputs["ff1_w"], np.float32)
    ff2 = np.asarray(inputs["ff2_w"], np.float32)
    ow = np.asarray(inputs["out_w"], np.float32)

    # contiguous device layouts
    # wqp[l, p, kt, o] = Mq[l, o, kt*128 + p]
    wqp_p = np.ascontiguousarray(Mq.transpose(0, 2, 1).reshape(
        L, KT, 128, H * D).transpose(0, 2, 1, 3)).astype(np.float16)
    wv_p = np.ascontiguousarray(wv.reshape(L, KT, 128, H * D).transpose(
        0, 2, 1, 3)).astype(np.float16)
    ow_p = np.ascontiguousarray(
        ow.reshape(L, 12, 128, D).transpose(0, 2, 1, 3)).astype(np.float16)
    ff1_p = np.ascontiguousarray(
        ff1.reshape(L, KT, 128, DFF).transpose(0, 2, 1, 3)).astype(np.float16)
    ff2_p = np.ascontiguousarray(
        ff2.reshape(L, NMID, 128, D).transpose(0, 2, 1, 3)).astype(np.float16)

    smalls = np.zeros((L, 128, 32), np.float32)
    smalls[:, :, 0:12] = uq.reshape(L, 12, 128).transpose(0, 2, 1)
    smalls[:, :, 12:14] = obe.reshape(L, 2, 128).transpose(0, 2, 1)
    smalls[:, :, 14:22] = np.asarray(inputs["ff1_b"], np.float32).reshape(
        L, 8, 128).transpose(0, 2, 1)
    smalls[:, :, 22:24] = np.asarray(inputs["ff2_b"], np.float32).reshape(
        L, 2, 128).transpose(0, 2, 1)
    for ci, nm in ((24, "ln1_g"), (26, "ln1_b"), (28, "ln2_g"), (30, "ln2_b")):
        smalls[:, :, ci:ci + 2] = np.asarray(inputs[nm], np.float32).reshape(
            L, 2, 128).transpose(0, 2, 1)

    h0 = tok_emb[tokens] + pos_emb[None]                     # [B, S, D] f32
    maskbias = np.where(tokens == 0, np.float32(-1e9), np.float32(0.0))

    shared = {"wqp": wqp_p, "wv": wv_p, "ow": ow_p, "ff1": ff1_p,
              "ff2": ff2_p, "smalls": smalls}
    in_maps = []
    for c in range(N_CORES):
        hc = h0[c * BS:(c + 1) * BS].reshape(T, D)           # [800, 256]
        h0_dim = np.ascontiguousarray(hc.T.reshape(KT, 128, T)).astype(np.float16)
        mb = maskbias[c * BS:(c + 1) * BS]                   # [4, 200]
        maskT = np.full((128, 2 * BS), np.float32(-1e9))
        for b in range(BS):
            maskT[0:128, 2 * b] = mb[b, 0:128]
            maskT[0:S - 128, 2 * b + 1] = mb[b, 128:S]
        m = dict(shared)
        m["h0"] = h0_dim
        m["maskT"] = np.ascontiguousarray(maskT)
        in_maps.append(m)

    if "nc" not in _CACHE:
        _CACHE["nc"] = _build_program()
    nc = _CACHE["nc"]
    _CACHE["in_maps"] = in_maps

    res = run_bass_kernel_spmd(nc, in_maps, list(range(N_CORES)))
    out = np.concatenate(
        [res.results[c]["out"].astype(np.float32).T.reshape(BS, S, D)
         for c in range(N_CORES)], axis=0)
    return out


if __name__ == "__main__":
    data = np.load("/tmp/ref_data.npz")
    inputs = {k: data[k] for k in data.files if k != "expected"}
    got = kernel(**inputs)
    exp = data["expected"]
    err = np.abs(got - exp).max() / np.abs(exp).max()
    print(f"Relative error: {err:.3e}")


# revision 11
# speedup vs baseline: 1.2290x; 1.2262x over previous
"""Trainium2 Bass kernel for nn_BERT_tensor — instruction-minimized redesign.

Strategy vs baseline:
  - Data-parallel over batch: 4 seqs (800 tokens) per core x 8 cores.
  - TN (MPO) contraction folded into dense QKV weights on host.
  - Everything dim-major [256, 800]; NO PE transposes anywhere.
  - Attention computed transposed: scoresT[k,q] = K^T Q per (seq, head) with
    pad-mask as per-partition activation bias; exp in bf16 (fp32-range
    exponent, no max-subtraction needed: |scores| <= ~35); softmax
    denominator via ones-vector matmul (partition reduction on PE);
    normalization folded into the PSUM->SBUF move of ctx.
  - LayerNorm dim-major: mean/var via ones-matmul partition reductions,
    per-token stats broadcast back via rank-1 matmul; gamma/beta are
    per-partition scalars (free with ACT bias / DVE scalar ops).
  - Single fp16 h state (no separate f32 residual copy).
  - All weights host-prepacked so every DMA is contiguous; 8 DMAs/layer.
"""
import numpy as np
from contextlib import ExitStack

import concourse.bass as bass
import concourse.bacc as bacc
import concourse.tile as tile
import concourse.mybir as mybir
import concourse.bass_isa as bass_isa
from concourse.bass_utils import run_bass_kernel_spmd

dt = mybir.dt
AF = mybir.ActivationFunctionType
ALU = mybir.AluOpType

B, S, D = 32, 200, 256
H, DFF, VOCAB, L, TD = 6, 1024, 3500, 8, 2
N_CORES = 8
BS = B // N_CORES            # 4 seqs per core
T = BS * S                   # 800 tokens per core
KT = D // 128                # 2 tiles over emb dim
NQK = 24                     # m-tiles over Q|K outdim (3072)
NMID = DFF // 128            # 8 tiles over ffn hidden
CH = [(0, 512), (512, 288)]  # bank-aligned free chunks of 800
CH3 = [(0, 512), (512, 512), (1024, 176)]  # chunks of 1200
SEQT = [(0, 128), (128, 72)]
EPS = 1e-6

import os
L_RUN = int(os.environ.get("BERT_L_RUN", str(L)))
REP = int(os.environ.get("BERT_REP", "1"))

f16 = dt.float16
bf16 = dt.bfloat16
f32 = dt.float32

_CACHE = {}


def _build_program():
    nc = bacc.Bacc("TRN2", target_bir_lowering=False, debug=False,
                   num_devices=N_CORES)
    inp = {}

    def din(name, shape, dty):
        inp[name] = nc.dram_tensor(name, list(shape), dty, kind="ExternalInput").ap()
        return inp[name]

    h0_d = din("h0", [KT, 128, T], f16)
    mask_d = din("maskT", [128, 2 * BS], f32)
    wm_d = din("wm", [L, 128, 13312], f16)
    OQP, OWV, OOW, OF1, OF2 = 0, 3072, 6144, 9216, 11264
    sm_d = din("smalls", [L, 128, 32], f32)
    out_d = nc.dram_tensor("out", [D, T], f16, kind="ExternalOutput").ap()

    # smalls column map
    C_BQK, C_OBE, C_F1B, C_F2B = 0, 12, 14, 22
    C_G1, C_B1, C_G2, C_B2 = 24, 26, 28, 30

    with tile.TileContext(nc) as tc:
        with ExitStack() as ctx:
            cpool = ctx.enter_context(tc.tile_pool(name="const", bufs=1))
            wpool = ctx.enter_context(tc.tile_pool(name="weights", bufs=1))
            apool = ctx.enter_context(tc.tile_pool(name="acts", bufs=1))
            psmm = ctx.enter_context(tc.tile_pool(name="psmm", bufs=2, space="PSUM"))

            mask_t = cpool.tile([128, 2 * BS], f32, tag="mask", name="mask_t")
            nc.sync.dma_start(mask_t[:], mask_d[:])
            eps_t = cpool.tile([128, 1], f32, tag="eps", name="eps_t")
            nc.vector.memset(eps_t[:], EPS)

            def mmslot(name, shape=(128, 1536), dty=f32):
                return psmm.tile(list(shape), dty, tag="mm", name=name)

            def arslot(name):
                return apool.tile([128, H * S], f32, tag="ar", bufs=2, name=name)

            for rep in range(REP):
              h16 = []
              for kt in range(KT):
                  t = apool.tile([128, T], f16, tag="h", bufs=4,
                                 name=f"h_init{rep}_{kt}")
                  nc.sync.dma_start(t[:], h0_d[kt])
                  h16.append(t)

              for l in range(L_RUN):
                # ---- layer weights (contiguous DMAs) ----
                wm = wpool.tile([128, 13312], f16, tag="wm", bufs=2,
                                name=f"wm{rep}_{l}")
                nc.sync.dma_start(wm[:], wm_d[l])
                sm = wpool.tile([128, 32], f32, tag="sm", bufs=2,
                                name=f"sm{rep}_{l}")
                nc.sync.dma_start(sm[:], sm_d[l])

                # ---- Q' projection (dim-major fp16 [1536, 800]);
                # K is h itself: scoresT = h^T (M h + u) with M = Wk Wq^T
                qk = [apool.tile([128, H, T], f16, tag=f"qk{kt}", bufs=1,
                                 name=f"qk{rep}_{l}_{kt}") for kt in range(KT)]
                for m in range(12):
                    h, kto = m // KT, m % KT
                    ps = mmslot(f"psqk{rep}_{l}_{m}")
                    for kt in range(KT):
                        for (o, w) in CH:
                            nc.tensor.matmul(
                                ps[:, o:o + w],
                                wm[:, OQP + kt * 1536 + m * 128:
                                      OQP + kt * 1536 + (m + 1) * 128],
                                h16[kt][:, o:o + w],
                                start=(kt == 0), stop=(kt == KT - 1))
                    nc.scalar.activation(qk[kto][:, h, :], ps[:, 0:T],
                                         AF.Identity,
                                         bias=sm[:, C_BQK + m:C_BQK + m + 1])

                # ---- attention per sequence ----
                ctxA = apool.tile([128, 12, T], f16, tag="ctx", bufs=1,
                                  name=f"ctx{rep}_{l}")
                for b in range(BS):
                    # V token-major bf16 [ts, 1536] per seq-tile
                    vt = []
                    for ti, (to, ts) in enumerate(SEQT):
                        ps = mmslot(f"psv{rep}_{l}_{b}_{ti}")
                        for kt in range(KT):
                            for c in range(3):
                                nc.tensor.matmul(
                                    ps[0:ts, c * 512:(c + 1) * 512],
                                    h16[kt][:, b * S + to:b * S + to + ts],
                                    wm[:, OWV + kt * 1536 + c * 512:
                                          OWV + kt * 1536 + (c + 1) * 512],
                                    start=(kt == 0), stop=(kt == KT - 1))
                        v = apool.tile([128, H * D], bf16, tag="v", bufs=4,
                                       name=f"v{rep}_{l}_{b}_{ti}")
                        nc.scalar.activation(v[0:ts, :], ps[0:ts, 0:H * D], AF.Copy)
                        vt.append(v)

                    # scoresT + exp (bf16), head-packed PSUM [128, 6, 256]
                    expT = []
                    for ti, (ko, ks) in enumerate(SEQT):
                        ps = psmm.tile([128, H, 256], f32, tag="mm",
                                       name=f"pssc{rep}_{l}_{b}_{ti}")
                        for kt in range(KT):
                            for hp in range(3):
                                nc.tensor.matmul(
                                    ps[0:ks, 2 * hp:2 * hp + 2, 0:S],
                                    h16[kt][:, b * S + ko:b * S + ko + ks],
                                    qk[kt][:, 2 * hp:2 * hp + 2,
                                           b * S:(b + 1) * S],
                                    start=(kt == 0), stop=(kt == KT - 1))
                        e = apool.tile([128, H, S], bf16, tag="expT", bufs=4,
                                       name=f"expT{rep}_{l}_{b}_{ti}")
                        nc.scalar.activation(
                            e[0:ks, :, :], ps[0:ks, :, 0:S], AF.Exp,
                            bias=mask_t[0:ks, b * 2 + ti:b * 2 + ti + 1])
                        expT.append(e)

                    # softmax denominators via gpsimd partition reductions
                    ar0 = arslot(f"ar0{rep}_{l}_{b}")
                    ar1 = arslot(f"ar1{rep}_{l}_{b}")
                    nc.gpsimd.partition_all_reduce(
                        ar0[:, :], expT[0][:, :, :], channels=128,
                        reduce_op=bass_isa.ReduceOp.add)
                    nc.gpsimd.partition_all_reduce(
                        ar1[0:72, :], expT[1][0:72, :, :], channels=72,
                        reduce_op=bass_isa.ReduceOp.add)
                    rs = apool.tile([1, H * S], f32, tag="rs", bufs=1,
                                    name=f"rs{rep}_{l}_{b}")
                    nc.vector.tensor_tensor(rs[0:1, :], ar0[0:1, :], ar1[0:1, :],
                                            op=ALU.add)
                    nc.vector.reciprocal(rs[0:1, :], rs[0:1, :])
                    rsb = apool.tile([128, H * S], f32, tag="rsb", bufs=1,
                                     name=f"rsb{rep}_{l}_{b}")
                    nc.gpsimd.partition_broadcast(rsb[:, :], rs[0:1, :])

                    # ctx dim-major, packed [128, 6, 256] per 3-head half;
                    # softmax normalization fused into the PSUM->SBUF move
                    for half in range(2):
                        pc = psmm.tile([128, 6, 256], f32, tag="mm",
                                       name=f"psctx{rep}_{l}_{b}_{half}")
                        for j in range(6):
                            h, dtile = half * 3 + j // 2, j % 2
                            for ti, (ko, ks) in enumerate(SEQT):
                                nc.tensor.matmul(
                                    pc[:, j, 0:S],
                                    vt[ti][0:ks, h * D + dtile * 128:
                                           h * D + (dtile + 1) * 128],
                                    expT[ti][0:ks, h, :],
                                    start=(ti == 0), stop=(ti == 1))
                        for dtile in range(2):
                            nc.vector.tensor_tensor(
                                ctxA[:, half * 6 + dtile:half * 6 + 6:2,
                                     b * S:(b + 1) * S],
                                pc[:, dtile:6:2, 0:S],
                                rsb[:, half * 3 * S:(half + 1) * 3 * S],
                                op=ALU.mult)

                # ---- out projection + residual -> x16 ----
                x16 = []
                for d2 in range(KT):
                    ps = mmslot(f"pso{rep}_{l}_{d2}")
                    for ct in range(12):
                        for (o, w) in CH:
                            nc.tensor.matmul(
                                ps[:, o:o + w],
                                wm[:, OOW + ct * 256 + d2 * 128:
                                      OOW + ct * 256 + (d2 + 1) * 128],
                                ctxA[:, ct, o:o + w],
                                start=(ct == 0), stop=(ct == 11))
                    xt = apool.tile([128, T], f16, tag="x", bufs=4,
                                    name=f"x{rep}_{l}_{d2}")
                    nc.vector.scalar_tensor_tensor(
                        xt[:], ps[:, 0:T],
                        sm[:, C_OBE + d2:C_OBE + d2 + 1], h16[d2][:],
                        op0=ALU.add, op1=ALU.add)
                    x16.append(xt)

                def layer_norm(xt, gc, bc, tag, otag, obufs):
                    """dim-major LN: xt 2 fp16 [128,800] tiles -> 2 fp16 tiles."""
                    # -mean
                    # channel sums of x via gpsimd all-reduce (broadcast out)
                    arm = []
                    for kt in range(KT):
                        a = arslot(f"{tag}arm{rep}_{l}_{kt}")
                        nc.gpsimd.partition_all_reduce(
                            a[:, 0:T], xt[kt][:, :], channels=128,
                            reduce_op=bass_isa.ReduceOp.add)
                        arm.append(a)
                    xc, sq = [], []
                    for kt in range(KT):
                        t1 = apool.tile([128, T], f16, tag="lnxc", bufs=4,
                                        name=f"{tag}t1{rep}_{l}_{kt}")
                        nc.vector.scalar_tensor_tensor(
                            t1[:], arm[0][:, 0:T], -1.0 / D, xt[kt][:],
                            op0=ALU.mult, op1=ALU.add)
                        c = apool.tile([128, T], f16, tag="lnxc", bufs=4,
                                       name=f"{tag}xc{rep}_{l}_{kt}")
                        nc.vector.scalar_tensor_tensor(
                            c[:], arm[1][:, 0:T], -1.0 / D, t1[:],
                            op0=ALU.mult, op1=ALU.add)
                        xc.append(c)
                        s = apool.tile([128, T], f16, tag="lnsq", bufs=2,
                                       name=f"{tag}sq{rep}_{l}_{kt}")
                        nc.scalar.activation(s[:], c[:], AF.Square)
                        sq.append(s)
                    # rstd = 1/sqrt(var+eps), broadcast across partitions
                    arv = []
                    for kt in range(KT):
                        a = arslot(f"{tag}arv{rep}_{l}_{kt}")
                        nc.gpsimd.partition_all_reduce(
                            a[:, 0:T], sq[kt][:, :], channels=128,
                            reduce_op=bass_isa.ReduceOp.add)
                        arv.append(a)
                    var = apool.tile([128, T], f32, tag="lnvar", bufs=1,
                                     name=f"{tag}var{rep}_{l}")
                    nc.vector.tensor_tensor(var[:], arv[0][:, 0:T], arv[1][:, 0:T],
                                            op=ALU.add)
                    nc.scalar.activation(var[:], var[:], AF.Sqrt, scale=1.0 / D,
                                         bias=eps_t[:, :])
                    nc.vector.reciprocal(var[:], var[:])
                    outs = []
                    for kt in range(KT):
                        tmp = apool.tile([128, T], f16, tag="lntmp", bufs=2,
                                         name=f"{tag}tmp{rep}_{l}_{kt}")
                        nc.vector.scalar_tensor_tensor(
                            tmp[:], xc[kt][:], sm[:, gc + kt:gc + kt + 1],
                            var[:], op0=ALU.mult, op1=ALU.mult)
                        o = apool.tile([128, T], f16, tag=otag, bufs=obufs,
                                       name=f"{tag}o{rep}_{l}_{kt}")
                        nc.scalar.activation(o[:], tmp[:], AF.Identity,
                                             bias=sm[:, bc + kt:bc + kt + 1])
                        outs.append(o)
                    return outs

                o1 = layer_norm(x16, C_G1, C_B1, "ln1", "o1", 2)

                # ---- FFN ----
                mid = []
                for m in range(NMID):
                    ps = mmslot(f"psf1{rep}_{l}_{m}")
                    for kt in range(KT):
                        for (o, w) in CH:
                            nc.tensor.matmul(
                                ps[:, o:o + w],
                                wm[:, OF1 + kt * 1024 + m * 128:
                                      OF1 + kt * 1024 + (m + 1) * 128],
                                o1[kt][:, o:o + w],
                                start=(kt == 0), stop=(kt == KT - 1))
                    mt = apool.tile([128, T], f16, tag="mid", bufs=NMID,
                                    name=f"mid{rep}_{l}_{m}")
                    nc.scalar.activation(mt[:], ps[:, 0:T], AF.Relu,
                                         bias=sm[:, C_F1B + m:C_F1B + m + 1])
                    mid.append(mt)

                x2 = []
                for d2 in range(KT):
                    ps = mmslot(f"psf2{rep}_{l}_{d2}")
                    for mt in range(NMID):
                        for (o, w) in CH:
                            nc.tensor.matmul(
                                ps[:, o:o + w],
                                wm[:, OF2 + mt * 256 + d2 * 128:
                                      OF2 + mt * 256 + (d2 + 1) * 128],
                                mid[mt][:, o:o + w],
                                start=(mt == 0), stop=(mt == NMID - 1))
                    xt = apool.tile([128, T], f16, tag="x", bufs=4,
                                    name=f"x2{rep}_{l}_{d2}")
                    nc.vector.scalar_tensor_tensor(
                        xt[:], ps[:, 0:T],
                        sm[:, C_F2B + d2:C_F2B + d2 + 1], o1[d2][:],
                        op0=ALU.add, op1=ALU.add)
                    x2.append(xt)

                h16 = layer_norm(x2, C_G2, C_B2, "ln2", "h", 4)

                if l == L_RUN - 1:
                    for kt in range(KT):
                        nc.sync.dma_start(out_d[kt * 128:(kt + 1) * 128, :],
                                          h16[kt][:])

    nc.compile()
    return nc


def _fold_weights(wqkv_w, wqkv_b, A1, A2, A3, A4, tnb, out_w, out_b):
    """Fold TN contraction into dense weights; fold v-bias into out bias;
    fold 1/sqrt(D) into Q."""
    wqkv_w = np.asarray(wqkv_w, np.float32)
    wqkv_b = np.asarray(wqkv_b, np.float32)
    out_w = np.asarray(out_w, np.float32)
    out_b = np.asarray(out_b, np.float32)
    tnb = np.asarray(tnb, np.float32)
    scale = 1.0 / np.sqrt(np.float32(D))

    W_full = np.zeros((L, 3, D, H * D), np.float32)
    b_full = np.zeros((L, 3, H * D), np.float32)
    for l in range(L):
        for x in range(3):
            wt = np.einsum('pmi,qmnj,rnok,tol->pqrtijkl',
                           np.asarray(A1[l, x], np.float64),
                           np.asarray(A2[l, x], np.float64),
                           np.asarray(A3[l, x], np.float64),
                           np.asarray(A4[l, x], np.float64),
                           optimize=True).reshape(D, 4 * D).astype(np.float32)
            W_full[l, x] = np.concatenate([wqkv_w[l, x], wt], axis=1)
            b_full[l, x] = np.concatenate([wqkv_b[l, x], tnb[l, x]])
    W_full[:, 0] *= scale
    b_full[:, 0] *= scale

    # fold K into Q: scoresT[k,q] = h_k . (M h_q + u) with
    # M = Wk Wq^T (per head), u = Wk bq; per-q softmax constants dropped
    Mq = np.zeros((L, H * D, D), np.float32)     # [L, 1536(out), 256(emb)]
    u = np.zeros((L, H * D), np.float32)
    for l in range(L):
        for h in range(H):
            s = slice(h * D, (h + 1) * D)
            wq = W_full[l, 0][:, s].astype(np.float64)   # [256e, 256j]
            wk = W_full[l, 1][:, s].astype(np.float64)
            Mq[l, s, :] = (wk @ wq.T).astype(np.float32)  # [256d, 256e]
            u[l, s] = (wk @ b_full[l, 0][s].astype(np.float64)).astype(np.float32)
    wv = W_full[:, 2]                                            # [L, 256, 1536]
    bv = b_full[:, 2]
    obe = out_b + np.einsum('lc,lcd->ld', bv, out_w)             # [L, 256]
    return Mq, u, wv, obe


def kernel(**inputs):
    tokens = np.asarray(inputs["tokens"])
    tok_emb = np.asarray(inputs["tok_emb"], np.float32)
    pos_emb = np.asarray(inputs["pos_emb"], np.float32)

    Mq, uq, wv, obe = _fold_weights(
        inputs["wqkv_w"], inputs["wqkv_b"], inputs["A1"], inputs["A2"],
        inputs["A3"], inputs["A4"], inputs["tnb"], inputs["out_w"],
        inputs["out_b"])
    ff1 = np.asarray(inputs["ff1_w"], np.float32)
    ff2 = np.asarray(inputs["ff2_w"], np.float32)
    ow = np.asarray(inputs["out_w"], np.float32)

    # contiguous device layouts, all packed into one [128, 13312] slab
    wqp_p = Mq.transpose(0, 2, 1).reshape(
        L, KT, 128, H * D).transpose(0, 2, 1, 3).reshape(L, 128, -1)
    wv_p = wv.reshape(L, KT, 128, H * D).transpose(0, 2, 1, 3).reshape(
        L, 128, -1)
    ow_p = ow.reshape(L, 12, 128, D).transpose(0, 2, 1, 3).reshape(L, 128, -1)
    ff1_p = ff1.reshape(L, KT, 128, DFF).transpose(0, 2, 1, 3).reshape(
        L, 128, -1)
    ff2_p = ff2.reshape(L, NMID, 128, D).transpose(0, 2, 1, 3).reshape(
        L, 128, -1)
    wm_p = np.ascontiguousarray(np.concatenate(
        [wqp_p, wv_p, ow_p, ff1_p, ff2_p], axis=2)).astype(np.float16)

    smalls = np.zeros((L, 128, 32), np.float32)
    smalls[:, :, 0:12] = uq.reshape(L, 12, 128).transpose(0, 2, 1)
    smalls[:, :, 12:14] = obe.reshape(L, 2, 128).transpose(0, 2, 1)
    smalls[:, :, 14:22] = np.asarray(inputs["ff1_b"], np.float32).reshape(
        L, 8, 128).transpose(0, 2, 1)
    smalls[:, :, 22:24] = np.asarray(inputs["ff2_b"], np.float32).reshape(
        L, 2, 128).transpose(0, 2, 1)
    for ci, nm in ((24, "ln1_g"), (26, "ln1_b"), (28, "ln2_g"), (30, "ln2_b")):
        smalls[:, :, ci:ci + 2] = np.asarray(inputs[nm], np.float32).reshape(
            L, 2, 128).transpose(0, 2, 1)

    h0 = tok_emb[tokens] + pos_emb[None]                     # [B, S, D] f32
    maskbias = np.where(tokens == 0, np.float32(-1e9), np.float32(0.0))

    shared = {"wm": wm_p, "smalls": smalls}
    in_maps = []
    for c in range(N_CORES):
        hc = h0[c * BS:(c + 1) * BS].reshape(T, D)           # [800, 256]
        h0_dim = np.ascontiguousarray(hc.T.reshape(KT, 128, T)).astype(np.float16)
        mb = maskbias[c * BS:(c + 1) * BS]                   # [4, 200]
        maskT = np.full((128, 2 * BS), np.float32(-1e9))
        for b in range(BS):
            maskT[0:128, 2 * b] = mb[b, 0:128]
            maskT[0:S - 128, 2 * b + 1] = mb[b, 128:S]
        m = dict(shared)
        m["h0"] = h0_dim
        m["maskT"] = np.ascontiguousarray(maskT)
        in_maps.append(m)

    if "nc" not in _CACHE:
        _CACHE["nc"] = _build_program()
    nc = _CACHE["nc"]
    _CACHE["in_maps"] = in_maps

    res = run_bass_kernel_spmd(nc, in_maps, list(range(N_CORES)))
    out = np.concatenate(
        [res.results[c]["out"].astype(np.float32).T.reshape(BS, S, D)
         for c in range(N_CORES)], axis=0)
    return out


if __name__ == "__main__":
    data = np.load("/tmp/ref_data.npz")
    inputs = {k: data[k] for k in data.files if k != "expected"}
    got = kernel(**inputs)
    exp = data["expected"]
    err = np.abs(got - exp).max() / np.abs(exp).max()
    print(f"Relative error: {err:.3e}")
